# revision 18
# baseline (speedup 1.0000x reference)
"""DampingGCN Trainium2 kernel — 8-core SPMD, v2.

Math: 3x [h = relu(segsum(norm_e * h[src]) + b)], then h @ Wl + bl, where
norm_e = dis[src]*dis[dst] (gcn norm with self-loops). segsum commutes with
the dense transform, so each layer aggregates RAW features and applies W
after aggregation.

v2 design vs v1:
- One-hot scatter tiles are HOST-PRECOMPUTED in DRAM (bf16) with the gcn
  norm baked into the values; no on-chip one-hot build, no dis scaling.
- Layer 1 needs only x[src] (2 features): the per-edge message stream is
  host-built and DMA-streamed; layer 1 has NO gather at all.
- Scatter matmul uses lhsT=msg so psT comes out feature-major [64, 128],
  feeding the dense stage without the per-block transpose.
- Nodes are re-assigned to (core, block, slot) with a degree-balanced
  snake so per-(block,page) tile counts are near-ideal.
- Gather calls rotate over 4 SWDGE queues.

Per layer 2/3 (per core, dst-sharded 12500 nodes): table [N, 128] bf16
node-major in DRAM (cols 0:64 = activations); groups of dst-blocks; per
(group, page): gpsimd.dma_gather pulls 256B rows into msg SBUF; per tile:
matmul(psT[64,128] += msg[:,t,0:64]^T @ oh[:,t,:]); per block: evict psT,
dense W matmul + relu + bias, transpose back, write shard; AllGather.
"""

import numpy as np

N, E, H, C = 100000, 1000000, 64, 8
BLK = 128
NSH = N // C          # 12500
NBLK = (NSH + BLK - 1) // BLK   # 98
LASTB = NSH - (NBLK - 1) * BLK  # 84
# src pages sized so per-(block,page,core) cell counts stay under 3*128
PSTART = [0, 27500, 55000, 82500]
PSIZE = [27500, 27500, 27500, 17500]
NPG = 4
GT = 48               # tiles per gather group (SBUF msg+oh budget)


def _host_prep(x, edge_index):
    from ml_dtypes import bfloat16

    # degrees/norm INCLUDE self-loops (gcn_norm semantics), but self-loop
    # edges are handled on-chip via a scaled-identity matmul, so the edge
    # streams below hold only the E real edges.
    src0 = edge_index[0]
    dst0 = edge_index[1]
    deg = np.bincount(dst0, minlength=N).astype(np.float32) + 1.0
    dis = 1.0 / np.sqrt(deg)
    normv = (dis[src0] * dis[dst0]).astype(np.float32)

    # ---- degree-balanced snake assignment of nodes to (core, block, slot)
    # bins: (c, b) for c in 0..7, b in 0..97; capacity 128 (84 for b=97).
    order = np.argsort(-deg, kind="stable")
    nbins = C * NBLK
    caps = np.full(nbins, BLK, dtype=np.int64)
    caps[NBLK - 1::NBLK] = LASTB
    bin_of = np.empty(N, dtype=np.int64)
    fill = np.zeros(nbins, dtype=np.int64)
    pos = 0
    bi, step = 0, 1
    # snake over bins, skipping full ones
    for n in order:
        while fill[bi] >= caps[bi]:
            bi += step
            if bi == nbins:
                bi, step = nbins - 1, -1
            elif bi < 0:
                bi, step = 0, 1
        bin_of[n] = bi
        fill[bi] += 1
        bi += step
        if bi == nbins:
            bi, step = nbins - 1, -1
        elif bi < 0:
            bi, step = 0, 1
    # slot within bin
    slot = np.zeros(N, dtype=np.int64)
    binfill = np.zeros(nbins, dtype=np.int64)
    for n in order:
        slot[n] = binfill[bin_of[n]]
        binfill[bin_of[n]] += 1
    c_of = bin_of // NBLK
    b_of = bin_of % NBLK
    gid = c_of * NSH + b_of * BLK + slot          # new global id per node
    inv = np.empty(N, dtype=np.int64)
    inv[gid] = np.arange(N)

    gs = gid[src0]                                 # new ids per edge
    gd = gid[dst0]
    core = gd // NSH
    bl = (gd % NSH) // BLK
    doff = (gd % NSH) % BLK
    pg = np.searchsorted(np.asarray(PSTART[1:] + [N]), gs, side="right")

    # ---------- layer 1 tiling: cells = (core, block) ----------
    cnt1 = np.zeros((C, NBLK), dtype=np.int64)
    np.add.at(cnt1, (core, bl), 1)
    t1 = np.ceil(cnt1.max(axis=0) / BLK).astype(np.int64)      # [NBLK]
    T1 = int(t1.sum())
    start1 = np.zeros(NBLK, dtype=np.int64)
    start1[1:] = np.cumsum(t1)[:-1]

    # ---------- layer 2/3 tiling: cells = (block, page) ----------
    cnt2 = np.zeros((C, NBLK, NPG), dtype=np.int64)
    np.add.at(cnt2, (core, bl, pg), 1)
    t_bp = np.ceil(cnt2.max(axis=0) / BLK).astype(np.int64)    # [NBLK, NPG]
    blk_tiles = t_bp.sum(axis=1)

    groups = []
    cur, cur_t = [], 0
    for b in range(NBLK):
        if cur and cur_t + blk_tiles[b] > GT:
            groups.append(cur)
            cur, cur_t = [], 0
        cur.append(b)
        cur_t += blk_tiles[b]
    groups.append(cur)

    T = int(blk_tiles.sum())
    block_cols = [[] for _ in range(NBLK)]
    gp_ranges = []
    cell_start = np.zeros((NBLK, NPG), dtype=np.int64)
    col = 0
    for g in groups:
        rng = []
        for p in range(NPG):
            st = col
            for b in g:
                cell_start[b, p] = col
                for _ in range(int(t_bp[b, p])):
                    block_cols[b].append(col)
                    col += 1
            rng.append((p, st, col - st))
        gp_ranges.append(rng)
    assert col == T

    # ---------- per-core streams ----------
    xb = x.astype(bfloat16)
    oh1s, msg1s, idxs, oh2s = [], [], [], []
    for c in range(C):
        m = core == c
        s_c, b_c, d_c, p_c, nv = gs[m], bl[m], doff[m], pg[m], normv[m]

        # layer 1: rank within block
        o = np.argsort(b_c, kind="stable")
        sb, db, dofb, nvb, srb = b_c[o], s_c[o], d_c[o], nv[o], src0[m][o]
        # cumcount per block
        uq, first, cn = np.unique(sb, return_index=True, return_counts=True)
        rank = np.arange(len(sb)) - np.repeat(first, cn)
        pos1 = start1[sb] * BLK + rank
        oh1 = np.zeros((T1 * BLK, BLK), dtype=np.float32)
        oh1[pos1, dofb] = nvb
        oh1 = oh1.reshape(T1, BLK, BLK).transpose(1, 0, 2)     # [128, T1, 128]
        msg1 = np.zeros((T1 * BLK, 2), dtype=np.float32)
        msg1[pos1] = x[srb]
        msg1 = msg1.reshape(T1, BLK, 2).transpose(1, 0, 2)     # [128, T1, 2]
        oh1s.append(np.ascontiguousarray(oh1.astype(bfloat16)))
        msg1s.append(np.ascontiguousarray(msg1.astype(bfloat16)))

        # layer 2/3: rank within (block, page) cell
        key = b_c * NPG + p_c
        o2 = np.argsort(key, kind="stable")
        k2, s2, d2, nv2 = key[o2], s_c[o2], d_c[o2], nv[o2]
        uq, first, cn = np.unique(k2, return_index=True, return_counts=True)
        rank2 = np.arange(len(k2)) - np.repeat(first, cn)
        pos2 = cell_start[k2 // NPG, k2 % NPG] * BLK + rank2
        idxv = np.zeros(T * BLK, dtype=np.int16)
        pstart = np.asarray(PSTART, dtype=np.int64)
        idxv[pos2] = (s2 - pstart[k2 % NPG]).astype(np.int16)
        idx16 = np.tile(idxv.reshape(-1, 16).T, (8, 1))        # [128, T*8]
        oh2 = np.zeros((T * BLK, BLK), dtype=np.float32)
        oh2[pos2, d2] = nv2
        oh2 = oh2.reshape(T, BLK, BLK).transpose(1, 0, 2)
        idxs.append(np.ascontiguousarray(idx16))
        oh2s.append(np.ascontiguousarray(oh2.astype(bfloat16)))

    # per-core self-term arrays: dis^2 and x in new node order, wrapped
    dis2s, xshs = [], []
    pad = NBLK * BLK - NSH
    for c in range(C):
        nodes = inv[c * NSH:(c + 1) * NSH]
        d2v = np.concatenate([dis[nodes] ** 2, np.zeros(pad, np.float32)])
        dis2s.append(np.ascontiguousarray(
            d2v.reshape(NBLK, BLK).T.astype(np.float32)))      # [128, NBLK]
        xv = np.concatenate([x[nodes], np.zeros((pad, 2), np.float32)])
        xshs.append(np.ascontiguousarray(
            xv.reshape(NBLK, BLK, 2).transpose(1, 0, 2).astype(np.float32)))

    struct = dict(T=T, T1=T1, t1=t1, start1=start1, groups=groups,
                  gp_ranges=gp_ranges, block_cols=block_cols, t_bp=t_bp)
    data = dict(oh1=oh1s, msg1=msg1s, idx=idxs, oh2=oh2s, inv=inv,
                dis2=dis2s, xsh=xshs)
    return struct, data


def _build(struct):
    from contextlib import ExitStack
    import concourse.bacc as bacc
    import concourse.mybir as mybir
    import concourse.tile as tile
    from concourse.masks import make_identity

    f32 = mybir.dt.float32
    bf16 = mybir.dt.bfloat16
    i16 = mybir.dt.int16
    T = struct["T"]
    T1 = struct["T1"]
    t1 = struct["t1"]
    start1 = struct["start1"]
    groups = struct["groups"]
    gp_ranges = struct["gp_ranges"]
    block_cols = struct["block_cols"]

    nc = bacc.Bacc("TRN2", target_bir_lowering=False, debug=False,
                   num_devices=C, num_swdge_queues=4)

    p_oh1 = nc.declare_dram_parameter("oh1", [128, T1, 128], bf16, isOutput=False)
    p_msg1 = nc.declare_dram_parameter("msg1", [128, T1, 2], bf16, isOutput=False)
    p_idx = nc.declare_dram_parameter("idx", [128, T * 8], i16, isOutput=False)
    p_oh2 = nc.declare_dram_parameter("oh2", [128, T, 128], bf16, isOutput=False)
    p_dis2 = nc.declare_dram_parameter("dis2", [128, NBLK], f32, isOutput=False)
    p_xsh = nc.declare_dram_parameter("xsh", [128, NBLK, 2], f32, isOutput=False)
    p_W = [nc.declare_dram_parameter(n, s, f32, isOutput=False) for n, s in
           [("W1", [2, H]), ("W2", [H, H]), ("W3", [H, H]), ("Wl", [H, 1])]]
    p_b = [nc.declare_dram_parameter(n, [H, 1], f32, isOutput=False) for n in
           ["b1", "b2", "b3"]]
    p_bl = nc.declare_dram_parameter("bl", [1, 1], f32, isOutput=False)
    p_out = nc.declare_dram_parameter("out", [NSH, 1], f32, isOutput=True)

    table2 = nc.dram_tensor("table2", [N, 2 * H], bf16, addr_space="Shared")
    table3 = nc.dram_tensor("table3", [N, 2 * H], bf16, addr_space="Shared")
    shard2 = nc.dram_tensor("shard2", [NSH, 2 * H], bf16)
    shard3 = nc.dram_tensor("shard3", [NSH, 2 * H], bf16)

    with tile.TileContext(nc) as tc, ExitStack() as ctx:
        res = ctx.enter_context(tc.tile_pool(name="res", bufs=1))
        sb = ctx.enter_context(tc.tile_pool(name="sb", bufs=3))
        msgp = ctx.enter_context(tc.tile_pool(name="msgp", bufs=2))
        ohp = ctx.enter_context(tc.tile_pool(name="ohp", bufs=2))
        l1p = ctx.enter_context(tc.tile_pool(name="l1p", bufs=2))
        psT = ctx.enter_context(tc.tile_pool(name="psT", bufs=2, space="PSUM"))
        psD = ctx.enter_context(tc.tile_pool(name="psD", bufs=2, space="PSUM"))
        psX = ctx.enter_context(tc.tile_pool(name="psX", bufs=2, space="PSUM"))

        ident = res.tile([H, H], bf16)
        make_identity(nc, ident[:])
        ident128 = res.tile([128, 128], bf16)
        make_identity(nc, ident128[:])

        idx_s = res.tile([128, T * 8], i16)
        nc.sync.dma_start(out=idx_s[:], in_=p_idx[:])
        dis2_s = res.tile([128, NBLK], f32)
        nc.sync.dma_start(out=dis2_s[:], in_=p_dis2[:])
        xsh_s = res.tile([128, NBLK, 2], f32)
        nc.sync.dma_start(out=xsh_s[:], in_=p_xsh[:])

        Wt = [res.tile([2, H], f32, name="W1"), res.tile([H, H], f32, name="W2"),
              res.tile([H, H], f32, name="W3")]
        for t, p in zip(Wt, p_W[:3]):
            nc.sync.dma_start(out=t[:], in_=p[:])
        Wlt = res.tile([H, 1], bf16, name="Wl")
        wl32 = res.tile([H, 1], f32)
        nc.sync.dma_start(out=wl32[:], in_=p_W[3][:])
        nc.vector.tensor_copy(out=Wlt[:], in_=wl32[:])
        bt = [res.tile([H, 1], f32, name=f"b{i}") for i in range(3)]
        for t, p in zip(bt, p_b):
            nc.sync.dma_start(out=t[:], in_=p[:])
        blt = res.tile([1, 1], f32)
        nc.sync.dma_start(out=blt[:], in_=p_bl[:])

        def dense_and_store(li, b, pt):
            """pt: PSUM f32 [H, 128] (rows 0:F valid) feature-major block."""
            F = 2 if li == 0 else H
            rows = LASTB if b == NBLK - 1 else 128
            sT = sb.tile([H, 128], f32, tag="sT")
            nc.scalar.activation(out=sT[0:F, :], in_=pt[0:F, :],
                                 func=mybir.ActivationFunctionType.Copy)
            pu = psD.tile([H, 128], f32, tag="pu")
            nc.tensor.matmul(out=pu[:], lhsT=Wt[li][:], rhs=sT[0:F, :],
                             start=True, stop=True)
            hT = sb.tile([H, 128], bf16, tag="hT")
            nc.scalar.activation(out=hT[:], in_=pu[:],
                                 func=mybir.ActivationFunctionType.Relu,
                                 bias=bt[li][:, 0:1])
            if li < 2:
                pb = psX.tile([128, H], bf16, tag="pb")
                nc.tensor.transpose(out=pb[:], in_=hT[:], identity=ident[:])
                tn = sb.tile([128, H], bf16, tag="tn")
                nc.vector.tensor_copy(out=tn[:], in_=pb[:])
                shard = shard2 if li == 0 else shard3
                nc.sync.dma_start(
                    out=shard[b * 128:b * 128 + rows, 0:H],
                    in_=tn[0:rows, :])
            else:
                po = psD.tile([H, 128], f32, tag="pu", name="po")
                nc.tensor.matmul(out=po[0:1, :], lhsT=Wlt[:], rhs=hT[:],
                                 start=True, stop=True)
                ob = sb.tile([1, 128], f32, tag="ob")
                nc.scalar.activation(out=ob[:], in_=po[0:1, :],
                                     func=mybir.ActivationFunctionType.Identity,
                                     bias=blt[:, 0:1])
                nc.sync.dma_start(
                    out=p_out[b * 128:b * 128 + rows, :].rearrange("a c -> c a"),
                    in_=ob[:, 0:rows])

        # ================= layer 1 (no gather) =================
        T1MAX = int(t1.max())
        for b in range(NBLK):
            nt = int(t1[b])
            s0 = int(start1[b])
            ohc = l1p.tile([128, T1MAX, 128], bf16, tag="oh1c", name="ohc")
            nc.sync.dma_start(out=ohc[:, 0:nt, :], in_=p_oh1[:, s0:s0 + nt, :])
            mgc = l1p.tile([128, T1MAX, 2], bf16, tag="m1c", name="mgc")
            nc.sync.dma_start(out=mgc[:, 0:nt, :], in_=p_msg1[:, s0:s0 + nt, :])
            # self-loop term: pt = (dis2 * x_own)^T @ I
            xs = sb.tile([128, 2], bf16, tag="xs")
            nc.scalar.activation(out=xs[:], in_=xsh_s[:, b, :],
                                 func=mybir.ActivationFunctionType.Copy,
                                 scale=dis2_s[:, b:b + 1])
            pt = psT.tile([H, 128], f32, tag="psT", name="pt")
            nc.tensor.matmul(out=pt[0:2, :], lhsT=xs[:], rhs=ident128[:],
                             start=True, stop=False)
            for k in range(nt):
                nc.tensor.matmul(out=pt[0:2, :], lhsT=mgc[:, k, :],
                                 rhs=ohc[:, k, :],
                                 start=False, stop=(k == nt - 1))
            dense_and_store(0, b, pt)

        nc.gpsimd.collective_compute(
            "AllGather", mybir.AluOpType.bypass,
            replica_groups=[list(range(C))],
            ins=[shard2.ap()], outs=[table2.ap()])

        # ================= layers 2 and 3 =================
        for li, tbl, shard in ((1, table2, shard2), (2, table3, shard3)):
            for gi, g in enumerate(groups):
                g0 = block_cols[g[0]][0]
                gn = sum(len(block_cols[b]) for b in g)
                msg = msgp.tile([128, GT, 2 * H], bf16, tag="msg")
                ohb = ohp.tile([128, GT, 128], bf16, tag="ohb")
                nc.sync.dma_start(out=ohb[:, 0:gn, :], in_=p_oh2[:, g0:g0 + gn, :])
                for (p, st, ncols) in gp_ranges[gi]:
                    for s0 in range(0, ncols, 64):
                        n0 = min(64, ncols - s0)
                        stt = st + s0
                        nc.gpsimd.dma_gather(
                            out_ap=msg[:, stt - g0:stt - g0 + n0, :],
                            in_ap=tbl[PSTART[p]:PSTART[p] + PSIZE[p], :],
                            idxs_ap=idx_s[:, stt * 8:(stt + n0) * 8],
                            num_idxs=n0 * 128,
                            num_idxs_reg=n0 * 128,
                            elem_size=2 * H,
                            single_packet=False,
                            queue_num=gi % 4,
                        )
                for b in g:
                    cols = block_cols[b]
                    rows = LASTB if b == NBLK - 1 else 128
                    own = sb.tile([128, H], bf16, tag="own")
                    nc.sync.dma_start(out=own[0:rows, :],
                                      in_=shard[b * 128:b * 128 + rows, 0:H])
                    ows = sb.tile([128, H], bf16, tag="ows")
                    nc.scalar.activation(out=ows[:], in_=own[:],
                                         func=mybir.ActivationFunctionType.Copy,
                                         scale=dis2_s[:, b:b + 1])
                    pt = psT.tile([H, 128], f32, tag="psT", name="pt")
                    nc.tensor.matmul(out=pt[:], lhsT=ows[:], rhs=ident128[:],
                                     start=True, stop=False)
                    for k, t in enumerate(cols):
                        nc.tensor.matmul(
                            out=pt[:], lhsT=msg[:, t - g0, 0:H],
                            rhs=ohb[:, t - g0, :],
                            start=False, stop=(k == len(cols) - 1))
                    dense_and_store(li, b, pt)
            if li == 1:
                nc.gpsimd.collective_compute(
                    "AllGather", mybir.AluOpType.bypass,
                    replica_groups=[list(range(C))],
                    ins=[shard3.ap()], outs=[table3.ap()])

    nc.compile()
    return nc


def kernel(**inputs):
    from concourse.bass_utils import run_bass_kernel_spmd

    x = np.asarray(inputs["x"], dtype=np.float32)
    edge_index = np.asarray(inputs["edge_index"]).astype(np.int64)
    struct, data = _host_prep(x, edge_index)
    nc = _build(struct)

    shared = dict(
        W1=np.asarray(inputs["W1"], np.float32),
        W2=np.asarray(inputs["W2"], np.float32),
        W3=np.asarray(inputs["W3"], np.float32),
        Wl=np.asarray(inputs["Wl"], np.float32),
        b1=np.asarray(inputs["b1"], np.float32).reshape(H, 1),
        b2=np.asarray(inputs["b2"], np.float32).reshape(H, 1),
        b3=np.asarray(inputs["b3"], np.float32).reshape(H, 1),
        bl=np.asarray(inputs["bl"], np.float32).reshape(1, 1),
    )
    in_maps = [dict(shared, oh1=data["oh1"][c], msg1=data["msg1"][c],
                    idx=data["idx"][c], oh2=data["oh2"][c],
                    dis2=data["dis2"][c], xsh=data["xsh"][c]) for c in range(C)]
    res = run_bass_kernel_spmd(nc, in_maps, list(range(C)), **_RUN_KWARGS)
    global _LAST_RESULT
    _LAST_RESULT = res
    out_new = np.concatenate([res.results[c]["out"] for c in range(C)], axis=0)
    # out_new[g] is the value for new slot g; inv[g] = original node id.
    inv = data["inv"]
    full = np.empty((N, 1), dtype=np.float32)
    full[inv] = out_new.astype(np.float32)
    return full


# test.py sets _RUN_KWARGS = {"trace": True, ...} to profile; harness uses {}.
_RUN_KWARGS: dict = {}
_LAST_RESULT = None


# revision 23
# speedup vs baseline: 1.2075x; 1.2075x over previous
"""DampingGCN Trainium2 kernel — 8-core SPMD, v2.

Math: 3x [h = relu(segsum(norm_e * h[src]) + b)], then h @ Wl + bl, where
norm_e = dis[src]*dis[dst] (gcn norm with self-loops). segsum commutes with
the dense transform, so each layer aggregates RAW features and applies W
after aggregation.

v2 design vs v1:
- One-hot scatter tiles are HOST-PRECOMPUTED in DRAM (bf16) with the gcn
  norm baked into the values; no on-chip one-hot build, no dis scaling.
- Layer 1 needs only x[src] (2 features): the per-edge message stream is
  host-built and DMA-streamed; layer 1 has NO gather at all.
- Scatter matmul uses lhsT=msg so psT comes out feature-major [64, 128],
  feeding the dense stage without the per-block transpose.
- Nodes are re-assigned to (core, block, slot) with a degree-balanced
  snake so per-(block,page) tile counts are near-ideal.
- Gather calls rotate over 4 SWDGE queues.

Per layer 2/3 (per core, dst-sharded 12500 nodes): table [N, 128] bf16
node-major in DRAM (cols 0:64 = activations); groups of dst-blocks; per
(group, page): gpsimd.dma_gather pulls 256B rows into msg SBUF; per tile:
matmul(psT[64,128] += msg[:,t,0:64]^T @ oh[:,t,:]); per block: evict psT,
dense W matmul + relu + bias, transpose back, write shard; AllGather.
"""

import numpy as np

N, E, H, C = 100000, 1000000, 64, 8
BLK = 128
NSH = N // C          # 12500
NBLK = (NSH + BLK - 1) // BLK   # 98
LASTB = NSH - (NBLK - 1) * BLK  # 84
# src pages sized so per-(block,page,core) cell counts stay under 3*128
PSTART = [0, 27500, 55000, 82500]
PSIZE = [27500, 27500, 27500, 17500]
NPG = 4
GT = 64               # tiles per gather group (SBUF msg+oh budget)


def _host_prep(x, edge_index):
    from ml_dtypes import bfloat16

    # degrees/norm INCLUDE self-loops (gcn_norm semantics), but self-loop
    # edges are handled on-chip via a scaled-identity matmul, so the edge
    # streams below hold only the E real edges.
    src0 = edge_index[0]
    dst0 = edge_index[1]
    deg = np.bincount(dst0, minlength=N).astype(np.float32) + 1.0
    dis = 1.0 / np.sqrt(deg)
    normv = (dis[src0] * dis[dst0]).astype(np.float32)

    # ---- degree-balanced snake assignment of nodes to (core, block, slot)
    # bins: (c, b) for c in 0..7, b in 0..97; capacity 128 (84 for b=97).
    order = np.argsort(-deg, kind="stable")
    nbins = C * NBLK
    caps = np.full(nbins, BLK, dtype=np.int64)
    caps[NBLK - 1::NBLK] = LASTB
    bin_of = np.empty(N, dtype=np.int64)
    fill = np.zeros(nbins, dtype=np.int64)
    pos = 0
    bi, step = 0, 1
    # snake over bins, skipping full ones
    for n in order:
        while fill[bi] >= caps[bi]:
            bi += step
            if bi == nbins:
                bi, step = nbins - 1, -1
            elif bi < 0:
                bi, step = 0, 1
        bin_of[n] = bi
        fill[bi] += 1
        bi += step
        if bi == nbins:
            bi, step = nbins - 1, -1
        elif bi < 0:
            bi, step = 0, 1
    # slot within bin
    slot = np.zeros(N, dtype=np.int64)
    binfill = np.zeros(nbins, dtype=np.int64)
    for n in order:
        slot[n] = binfill[bin_of[n]]
        binfill[bin_of[n]] += 1
    c_of = bin_of // NBLK
    b_of = bin_of % NBLK
    gid = c_of * NSH + b_of * BLK + slot          # new global id per node
    inv = np.empty(N, dtype=np.int64)
    inv[gid] = np.arange(N)

    gs = gid[src0]                                 # new ids per edge
    gd = gid[dst0]
    core = gd // NSH
    bl = (gd % NSH) // BLK
    doff = (gd % NSH) % BLK
    pg = np.searchsorted(np.asarray(PSTART[1:] + [N]), gs, side="right")

    # ---------- layer 1 tiling: cells = (core, block) ----------
    cnt1 = np.zeros((C, NBLK), dtype=np.int64)
    np.add.at(cnt1, (core, bl), 1)
    t1 = np.ceil(cnt1.max(axis=0) / BLK).astype(np.int64)      # [NBLK]
    T1 = int(t1.sum())
    start1 = np.zeros(NBLK, dtype=np.int64)
    start1[1:] = np.cumsum(t1)[:-1]

    # ---------- layer 2/3 tiling: cells = (block, page) ----------
    cnt2 = np.zeros((C, NBLK, NPG), dtype=np.int64)
    np.add.at(cnt2, (core, bl, pg), 1)
    t_bp = np.ceil(cnt2.max(axis=0) / BLK).astype(np.int64)    # [NBLK, NPG]
    blk_tiles = t_bp.sum(axis=1)

    groups = []
    cur, cur_t = [], 0
    for b in range(NBLK):
        if cur and cur_t + blk_tiles[b] > GT:
            groups.append(cur)
            cur, cur_t = [], 0
        cur.append(b)
        cur_t += blk_tiles[b]
    groups.append(cur)

    T = int(blk_tiles.sum())
    block_cols = [[] for _ in range(NBLK)]
    gp_ranges = []
    cell_start = np.zeros((NBLK, NPG), dtype=np.int64)
    col = 0
    for g in groups:
        rng = []
        for p in range(NPG):
            st = col
            for b in g:
                cell_start[b, p] = col
                for _ in range(int(t_bp[b, p])):
                    block_cols[b].append(col)
                    col += 1
            rng.append((p, st, col - st))
        gp_ranges.append(rng)
    assert col == T

    # ---------- per-core streams ----------
    xb = x.astype(bfloat16)
    oh1s, msg1s, idxs, oh2s = [], [], [], []
    for c in range(C):
        m = core == c
        s_c, b_c, d_c, p_c, nv = gs[m], bl[m], doff[m], pg[m], normv[m]

        # layer 1: rank within block
        o = np.argsort(b_c, kind="stable")
        sb, db, dofb, nvb, srb = b_c[o], s_c[o], d_c[o], nv[o], src0[m][o]
        # cumcount per block
        uq, first, cn = np.unique(sb, return_index=True, return_counts=True)
        rank = np.arange(len(sb)) - np.repeat(first, cn)
        pos1 = start1[sb] * BLK + rank
        oh1 = np.zeros((T1 * BLK, BLK), dtype=np.float32)
        oh1[pos1, dofb] = nvb
        oh1 = oh1.reshape(T1, BLK, BLK).transpose(1, 0, 2)     # [128, T1, 128]
        msg1 = np.zeros((T1 * BLK, 2), dtype=np.float32)
        msg1[pos1] = x[srb]
        msg1 = msg1.reshape(T1, BLK, 2).transpose(1, 0, 2)     # [128, T1, 2]
        oh1s.append(np.ascontiguousarray(oh1.astype(bfloat16)))
        msg1s.append(np.ascontiguousarray(msg1.astype(bfloat16)))

        # layer 2/3: rank within (block, page) cell
        key = b_c * NPG + p_c
        o2 = np.argsort(key, kind="stable")
        k2, s2, d2, nv2 = key[o2], s_c[o2], d_c[o2], nv[o2]
        uq, first, cn = np.unique(k2, return_index=True, return_counts=True)
        rank2 = np.arange(len(k2)) - np.repeat(first, cn)
        pos2 = cell_start[k2 // NPG, k2 % NPG] * BLK + rank2
        idxv = np.zeros(T * BLK, dtype=np.int16)
        pstart = np.asarray(PSTART, dtype=np.int64)
        idxv[pos2] = (s2 - pstart[k2 % NPG]).astype(np.int16)
        idx16 = np.tile(idxv.reshape(-1, 16).T, (8, 1))        # [128, T*8]
        oh2 = np.zeros((T * BLK, BLK), dtype=np.float32)
        oh2[pos2, d2] = nv2
        oh2 = oh2.reshape(T, BLK, BLK).transpose(1, 0, 2)
        idxs.append(np.ascontiguousarray(idx16))
        oh2s.append(np.ascontiguousarray(oh2.astype(bfloat16)))

    # per-core self-term arrays: dis^2 and x in new node order, wrapped
    dis2s, xshs = [], []
    pad = NBLK * BLK - NSH
    for c in range(C):
        nodes = inv[c * NSH:(c + 1) * NSH]
        d2v = np.concatenate([dis[nodes] ** 2, np.zeros(pad, np.float32)])
        dis2s.append(np.ascontiguousarray(
            d2v.reshape(NBLK, BLK).T.astype(np.float32)))      # [128, NBLK]
        xv = np.concatenate([x[nodes], np.zeros((pad, 2), np.float32)])
        xshs.append(np.ascontiguousarray(
            xv.reshape(NBLK, BLK, 2).transpose(1, 0, 2).astype(np.float32)))

    struct = dict(T=T, T1=T1, t1=t1, start1=start1, groups=groups,
                  gp_ranges=gp_ranges, block_cols=block_cols, t_bp=t_bp)
    data = dict(oh1=oh1s, msg1=msg1s, idx=idxs, oh2=oh2s, inv=inv,
                dis2=dis2s, xsh=xshs)
    return struct, data


def _build(struct):
    from contextlib import ExitStack
    import concourse.bacc as bacc
    import concourse.mybir as mybir
    import concourse.tile as tile
    from concourse.masks import make_identity

    f32 = mybir.dt.float32
    bf16 = mybir.dt.bfloat16
    i16 = mybir.dt.int16
    T = struct["T"]
    T1 = struct["T1"]
    t1 = struct["t1"]
    start1 = struct["start1"]
    groups = struct["groups"]
    gp_ranges = struct["gp_ranges"]
    block_cols = struct["block_cols"]

    nc = bacc.Bacc("TRN2", target_bir_lowering=False, debug=False,
                   num_devices=C, num_swdge_queues=4,
                   dynamic_dma_scratch_size=32768)

    p_oh1 = nc.declare_dram_parameter("oh1", [128, T1, 128], bf16, isOutput=False)
    p_msg1 = nc.declare_dram_parameter("msg1", [128, T1, 2], bf16, isOutput=False)
    p_idx = nc.declare_dram_parameter("idx", [128, T * 8], i16, isOutput=False)
    p_oh2 = nc.declare_dram_parameter("oh2", [128, T, 128], bf16, isOutput=False)
    p_dis2 = nc.declare_dram_parameter("dis2", [128, NBLK], f32, isOutput=False)
    p_xsh = nc.declare_dram_parameter("xsh", [128, NBLK, 2], f32, isOutput=False)
    p_W = [nc.declare_dram_parameter(n, s, f32, isOutput=False) for n, s in
           [("W1", [2, H]), ("W2", [H, H]), ("W3", [H, H]), ("Wl", [H, 1])]]
    p_b = [nc.declare_dram_parameter(n, [H, 1], f32, isOutput=False) for n in
           ["b1", "b2", "b3"]]
    p_bl = nc.declare_dram_parameter("bl", [1, 1], f32, isOutput=False)
    p_out = nc.declare_dram_parameter("out", [NSH, 1], f32, isOutput=True)

    table2 = nc.dram_tensor("table2", [N, 2 * H], bf16, addr_space="Shared")
    table3 = nc.dram_tensor("table3", [N, 2 * H], bf16, addr_space="Shared")
    shard2 = nc.dram_tensor("shard2", [NSH, 2 * H], bf16)
    shard3 = nc.dram_tensor("shard3", [NSH, 2 * H], bf16)

    with tile.TileContext(nc) as tc, ExitStack() as ctx:
        res = ctx.enter_context(tc.tile_pool(name="res", bufs=1))
        sb = ctx.enter_context(tc.tile_pool(name="sb", bufs=3))
        msgp = ctx.enter_context(tc.tile_pool(name="msgp", bufs=3))
        ohp = ctx.enter_context(tc.tile_pool(name="ohp", bufs=2))
        l1p = ctx.enter_context(tc.tile_pool(name="l1p", bufs=2))
        psT = ctx.enter_context(tc.tile_pool(name="psT", bufs=2, space="PSUM"))
        psD = ctx.enter_context(tc.tile_pool(name="psD", bufs=2, space="PSUM"))
        psX = ctx.enter_context(tc.tile_pool(name="psX", bufs=2, space="PSUM"))

        ident = res.tile([H, H], bf16)
        make_identity(nc, ident[:])
        ident128 = res.tile([128, 128], bf16)
        make_identity(nc, ident128[:])

        idx_s = res.tile([128, T * 8], i16)
        nc.sync.dma_start(out=idx_s[:], in_=p_idx[:])
        dis2_s = res.tile([128, NBLK], f32)
        nc.sync.dma_start(out=dis2_s[:], in_=p_dis2[:])
        xsh_s = res.tile([128, NBLK, 2], f32)
        nc.sync.dma_start(out=xsh_s[:], in_=p_xsh[:])

        Wt = [res.tile([2, H], f32, name="W1"), res.tile([H, H], f32, name="W2"),
              res.tile([H, H], f32, name="W3")]
        for t, p in zip(Wt, p_W[:3]):
            nc.sync.dma_start(out=t[:], in_=p[:])
        Wlt = res.tile([H, 1], bf16, name="Wl")
        wl32 = res.tile([H, 1], f32)
        nc.sync.dma_start(out=wl32[:], in_=p_W[3][:])
        nc.vector.tensor_copy(out=Wlt[:], in_=wl32[:])
        bt = [res.tile([H, 1], f32, name=f"b{i}") for i in range(3)]
        for t, p in zip(bt, p_b):
            nc.sync.dma_start(out=t[:], in_=p[:])
        blt = res.tile([1, 1], f32)
        nc.sync.dma_start(out=blt[:], in_=p_bl[:])

        def dense_and_store(li, b, pt):
            """pt: PSUM f32 [H, 128] (rows 0:F valid) feature-major block."""
            F = 2 if li == 0 else H
            rows = LASTB if b == NBLK - 1 else 128
            sT = sb.tile([H, 128], f32, tag="sT")
            nc.scalar.activation(out=sT[0:F, :], in_=pt[0:F, :],
                                 func=mybir.ActivationFunctionType.Copy)
            pu = psD.tile([H, 128], f32, tag="pu")
            nc.tensor.matmul(out=pu[:], lhsT=Wt[li][:], rhs=sT[0:F, :],
                             start=True, stop=True)
            hT = sb.tile([H, 128], bf16, tag="hT")
            nc.scalar.activation(out=hT[:], in_=pu[:],
                                 func=mybir.ActivationFunctionType.Relu,
                                 bias=bt[li][:, 0:1])
            if li < 2:
                pb = psX.tile([128, H], bf16, tag="pb")
                nc.tensor.transpose(out=pb[:], in_=hT[:], identity=ident[:])
                tn = sb.tile([128, H], bf16, tag="tn")
                nc.vector.tensor_copy(out=tn[:], in_=pb[:])
                shard = shard2 if li == 0 else shard3
                nc.sync.dma_start(
                    out=shard[b * 128:b * 128 + rows, 0:H],
                    in_=tn[0:rows, :])
            else:
                po = psD.tile([H, 128], f32, tag="pu", name="po")
                nc.tensor.matmul(out=po[0:1, :], lhsT=Wlt[:], rhs=hT[:],
                                 start=True, stop=True)
                ob = sb.tile([1, 128], f32, tag="ob")
                nc.scalar.activation(out=ob[:], in_=po[0:1, :],
                                     func=mybir.ActivationFunctionType.Identity,
                                     bias=blt[:, 0:1])
                nc.sync.dma_start(
                    out=p_out[b * 128:b * 128 + rows, :].rearrange("a c -> c a"),
                    in_=ob[:, 0:rows])

        # ================= layer 1 (no gather) =================
        T1MAX = int(t1.max())
        for b in range(NBLK):
            nt = int(t1[b])
            s0 = int(start1[b])
            ohc = l1p.tile([128, T1MAX, 128], bf16, tag="oh1c", name="ohc")
            nc.sync.dma_start(out=ohc[:, 0:nt, :], in_=p_oh1[:, s0:s0 + nt, :])
            mgc = l1p.tile([128, T1MAX, 2], bf16, tag="m1c", name="mgc")
            nc.sync.dma_start(out=mgc[:, 0:nt, :], in_=p_msg1[:, s0:s0 + nt, :])
            # self-loop term: pt = (dis2 * x_own)^T @ I
            xs = sb.tile([128, 2], bf16, tag="xs")
            nc.scalar.activation(out=xs[:], in_=xsh_s[:, b, :],
                                 func=mybir.ActivationFunctionType.Copy,
                                 scale=dis2_s[:, b:b + 1])
            pt = psT.tile([H, 128], f32, tag="psT", name="pt")
            nc.tensor.matmul(out=pt[0:2, :], lhsT=xs[:], rhs=ident128[:],
                             start=True, stop=False)
            for k in range(nt):
                nc.tensor.matmul(out=pt[0:2, :], lhsT=mgc[:, k, :],
                                 rhs=ohc[:, k, :],
                                 start=False, stop=(k == nt - 1))
            dense_and_store(0, b, pt)

        nc.gpsimd.collective_compute(
            "AllGather", mybir.AluOpType.bypass,
            replica_groups=[list(range(C))],
            ins=[shard2.ap()], outs=[table2.ap()])

        # ================= layers 2 and 3 =================
        for li, tbl, shard in ((1, table2, shard2), (2, table3, shard3)):
            for gi, g in enumerate(groups):
                g0 = block_cols[g[0]][0]
                gn = sum(len(block_cols[b]) for b in g)
                msg = msgp.tile([128, GT, 2 * H], bf16, tag="msg")
                ohb = ohp.tile([128, GT, 128], bf16, tag="ohb")
                nc.sync.dma_start(out=ohb[:, 0:gn, :], in_=p_oh2[:, g0:g0 + gn, :])
                for (p, st, ncols) in gp_ranges[gi]:
                    for s0 in range(0, ncols, 64):
                        n0 = min(64, ncols - s0)
                        stt = st + s0
                        nc.gpsimd.dma_gather(
                            out_ap=msg[:, stt - g0:stt - g0 + n0, :],
                            in_ap=tbl[PSTART[p]:PSTART[p] + PSIZE[p], :],
                            idxs_ap=idx_s[:, stt * 8:(stt + n0) * 8],
                            num_idxs=n0 * 128,
                            num_idxs_reg=n0 * 128,
                            elem_size=2 * H,
                            single_packet=False,
                            queue_num=gi % 4,
                        )
                for b in g:
                    cols = block_cols[b]
                    rows = LASTB if b == NBLK - 1 else 128
                    own = sb.tile([128, H], bf16, tag="own")
                    nc.sync.dma_start(out=own[0:rows, :],
                                      in_=shard[b * 128:b * 128 + rows, 0:H])
                    ows = sb.tile([128, H], bf16, tag="ows")
                    nc.scalar.activation(out=ows[:], in_=own[:],
                                         func=mybir.ActivationFunctionType.Copy,
                                         scale=dis2_s[:, b:b + 1])
                    pt = psT.tile([H, 128], f32, tag="psT", name="pt")
                    nc.tensor.matmul(out=pt[:], lhsT=ows[:], rhs=ident128[:],
                                     start=True, stop=False)
                    for k, t in enumerate(cols):
                        nc.tensor.matmul(
                            out=pt[:], lhsT=msg[:, t - g0, 0:H],
                            rhs=ohb[:, t - g0, :],
                            start=False, stop=(k == len(cols) - 1))
                    dense_and_store(li, b, pt)
            if li == 1:
                nc.gpsimd.collective_compute(
                    "AllGather", mybir.AluOpType.bypass,
                    replica_groups=[list(range(C))],
                    ins=[shard3.ap()], outs=[table3.ap()])

    nc.compile()
    return nc


def kernel(**inputs):
    from concourse.bass_utils import run_bass_kernel_spmd

    x = np.asarray(inputs["x"], dtype=np.float32)
    edge_index = np.asarray(inputs["edge_index"]).astype(np.int64)
    struct, data = _host_prep(x, edge_index)
    nc = _build(struct)

    shared = dict(
        W1=np.asarray(inputs["W1"], np.float32),
        W2=np.asarray(inputs["W2"], np.float32),
        W3=np.asarray(inputs["W3"], np.float32),
        Wl=np.asarray(inputs["Wl"], np.float32),
        b1=np.asarray(inputs["b1"], np.float32).reshape(H, 1),
        b2=np.asarray(inputs["b2"], np.float32).reshape(H, 1),
        b3=np.asarray(inputs["b3"], np.float32).reshape(H, 1),
        bl=np.asarray(inputs["bl"], np.float32).reshape(1, 1),
    )
    in_maps = [dict(shared, oh1=data["oh1"][c], msg1=data["msg1"][c],
                    idx=data["idx"][c], oh2=data["oh2"][c],
                    dis2=data["dis2"][c], xsh=data["xsh"][c]) for c in range(C)]
    res = run_bass_kernel_spmd(nc, in_maps, list(range(C)), **_RUN_KWARGS)
    global _LAST_RESULT
    _LAST_RESULT = res
    out_new = np.concatenate([res.results[c]["out"] for c in range(C)], axis=0)
    # out_new[g] is the value for new slot g; inv[g] = original node id.
    inv = data["inv"]
    full = np.empty((N, 1), dtype=np.float32)
    full[inv] = out_new.astype(np.float32)
    return full


# test.py sets _RUN_KWARGS = {"trace": True, ...} to profile; harness uses {}.
_RUN_KWARGS: dict = {}
_LAST_RESULT = None
